# revision 16
# baseline (speedup 1.0000x reference)
"""Banded local-attention kernel for Trainium2, 8 NeuronCores, raw-bass SPMD.

Math: reference computes attn = softmax(scores); attn_local = attn*mask;
attn_final = attn_local / ||attn_local||_2(row). The softmax denominator Z
cancels in the L2 normalization, so:
    attn_final[i,j] = mask[i,j]*exp(s[i,j]) / sqrt(sum_k mask[i,k]*exp(2*s[i,k]))
Only the banded window of scores (|i-j| <= 50) is ever needed.

Sharding: batch (16) over 8 cores, 2 batches/core. Per batch:
  x^T via PE transposes -> Q^T,K^T (transposed layout) + V (natural) via f32r
  matmuls -> banded scores per 128-row block -> exp/exp2 on ACT -> masked
  sum-of-squares + rsqrt scaling on DVE -> attn window written + zero strips
  -> A^T via PE transposes -> compound = A @ V.

Semaphore discipline (1 wait per instruction; DMA waits must equal the
total of all DMAs issued on that semaphore at satisfaction time):
  d_w: weight stage loads (serialized via dve gates) ; d_b: bq+bk ;
  d_m: masks ; d_x0/d_x1: per-batch x tiles ; dsw: bv broadcast (SWDGE) ;
  d_win[3]/d_comp[2]: per-SBUF-slot output DMAs ; d_strip: zero strips ;
  pe/dve/act: in-order engine progress ; pool_*: gpsimd objects.
"""
import sys
import numpy as np

sys.path.insert(0, "/opt/trn_rl_repo")

import concourse.bass as bass  # noqa: E402
from concourse import mybir  # noqa: E402
from concourse.bass_utils import run_bass_kernel_spmd  # noqa: E402

F32 = mybir.dt.float32
F32R = mybir.dt.float32r
AF = mybir.ActivationFunctionType
ALU = mybir.AluOpType

S = 1001
D = 512
P = 128
NB = 8
B_PER_CORE = 2
N_CORES = 8
SCALE = 1.0 / float(np.sqrt(512.0))

PB = [128] * 7 + [105]
I0 = [128 * b for b in range(NB)]
LOBLK = [max(0, b - 1) for b in range(NB)]
HIBLK = [min(NB - 1, b + 1) for b in range(NB)]
JBASE = [128 * LOBLK[b] for b in range(NB)]
JEND = [min(S, 128 * (HIBLK[b] + 1)) for b in range(NB)]
WB = [JEND[b] - JBASE[b] for b in range(NB)]
CHUNKS = [
    [(128 * (jb - LOBLK[b]), jb, PB[jb]) for jb in range(LOBLK[b], HIBLK[b] + 1)]
    for b in range(NB)
]
S_HALF = [(0, 512), (512, 490)]   # halves of padded S (1002) for f32r even-N
S_PAD = 1002
WE = [w + (w % 2) for w in WB]     # even-padded score window widths


def _counts():
    """Dry pass: cumulative semaphore counts for every labeled event."""
    cnt = {"pe": 0, "dve": 0, "act": 0,
           "win0": 0, "win1": 0, "win2": 0, "comp0": 0, "comp1": 0,
           "strip": 0}
    mark = {}

    def rec(sem, label, inc=1):
        cnt[sem] += inc
        mark[f"{sem}:{label}"] = cnt[sem]

    rec("dve", "wq")
    rec("dve", "wk")
    rec("dve", "wv")
    rec("dve", "pad0")
    rec("dve", "pad1")

    for bi in range(B_PER_CORE):
        for k in range(8):
            rec("pe", f"A{bi}_{k}")
        for k in range(8):
            rec("dve", f"cbA{bi}_{k}")
        for g in range(24):
            rec("pe", f"B{bi}_{g}")
        for g in range(24):
            rec("dve", f"cbB{bi}_{g}")
        rec("pe", f"sc{bi}_0")
        rec("pe", f"sc{bi}_1")
        for b in range(NB):
            rec("pe", f"at{bi}_{b}")
            rec("pe", f"cp{bi}_{b}")
            if b + 2 < NB:
                rec("pe", f"sc{bi}_{b+2}")
        for b in range(NB):
            rec("act", f"exp{bi}_{b}")
            rec("act", f"exp2{bi}_{b}")
            if b >= 1:
                rec("act", f"sqrt{bi}_{b-1}")
        rec("act", f"sqrt{bi}_{NB-1}")
        for b in range(NB + 2):
            if b >= 2:
                rec("dve", f"atcb{bi}_{b-2}")
                rec("dve", f"ccb{bi}_{b-2}")
            if b < NB:
                rec("dve", f"ttr{bi}_{b}")
            if 1 <= b <= NB:
                rec("dve", f"recip{bi}_{b-1}")
                rec("dve", f"stt{bi}_{b-1}")
        for b in range(NB):
            rec(f"win{b % 3}", f"win{bi}_{b}", 16)
            if JBASE[b] > 0:
                rec("strip", f"ls{bi}_{b}", 16)
            if JEND[b] < S:
                rec("strip", f"rs{bi}_{b}", 16)
            rec(f"comp{b % 2}", f"comp{bi}_{b}", 16)
    return cnt, mark


def build_program():
    cnt, mark = _counts()
    nc = bass.Bass()

    x_d = nc.declare_dram_parameter("x", [B_PER_CORE, S, D], F32, isOutput=False)
    wq_d = nc.declare_dram_parameter("Wq", [D, D], F32, isOutput=False)
    bq_d = nc.declare_dram_parameter("bq", [D], F32, isOutput=False)
    wk_d = nc.declare_dram_parameter("Wk", [D, D], F32, isOutput=False)
    bk_d = nc.declare_dram_parameter("bk", [D], F32, isOutput=False)
    wv_d = nc.declare_dram_parameter("Wv", [D, D], F32, isOutput=False)
    bv_d = nc.declare_dram_parameter("bv", [D], F32, isOutput=False)
    lm_d = nc.declare_dram_parameter("local_mat", [S, S], F32, isOutput=False)
    comp_d = nc.declare_dram_parameter("comp", [B_PER_CORE, S, D], F32, isOutput=True)
    attn_d = nc.declare_dram_parameter("attn", [B_PER_CORE, S, S], F32, isOutput=True)

    t = nc.alloc_sbuf_tensor
    w_stage = t("w_stage", [P, 4, D], F32).ap()
    wq_t = t("wq_t", [P, 4, D], F32R).ap()
    wk_t = t("wk_t", [P, 4, D], F32R).ap()
    wv_t = t("wv_t", [P, 4, D], F32R).ap()
    bq_t = t("bq_t", [P, 4], F32).ap()
    bk_t = t("bk_t", [P, 4], F32).ap()
    bv_b = t("bv_b", [P, D], F32).ap()
    mask_t = [t(f"mask{b}", [P, WB[b]], F32).ap() for b in range(NB)]
    zero_t = t("zero_t", [P, 768], F32).ap()
    ident = t("ident", [P, P], F32).ap()
    zbias = t("zbias", [P, 1], F32).ap()

    x_sb = [[t(f"x_{bi}_{sb}", [P, D], F32).ap() for sb in range(NB)]
            for bi in range(B_PER_CORE)]
    xT = [t(f"xT_{bi}", [P, 4, S_PAD], F32R).ap() for bi in range(B_PER_CORE)]
    qT = t("qT", [P, 4, S_PAD], F32R).ap()
    kT = t("kT", [P, 4, S_PAD], F32R).ap()
    v_t = t("v_t", [P, NB, D], F32R).ap()

    e_t = [t(f"e_{i}", [P, 384], F32).ap() for i in range(2)]
    e2_t = [t(f"e2_{i}", [P, 384], F32).ap() for i in range(2)]
    scr_t = [t(f"scr_{i}", [P, 384], F32).ap() for i in range(2)]
    a_t = [t(f"a_{i}", [P, 384], F32).ap() for i in range(3)]
    at_t = [t(f"at_{i}", [P, 384], F32R).ap() for i in range(2)]
    cs_t = [t(f"cs_{i}", [P, D], F32).ap() for i in range(2)]
    ssq_t = [t(f"ssq_{i}", [P, 1], F32).ap() for i in range(2)]
    r_t = [t(f"r_{i}", [P, 1], F32).ap() for i in range(2)]

    pb = [nc.alloc_psum_tensor(f"pb{i}", [P, D], F32).ap() for i in range(8)]

    dve_b_end = [mark["dve:ccb0_7"], mark["dve:ccb1_7"]]
    act_b_end = [mark["act:sqrt0_7"], mark["act:sqrt1_7"]]

    with (
        nc.Block() as block,
        nc.semaphore("d_w") as d_w,
        nc.semaphore("d_b") as d_b,
        nc.semaphore("d_m") as d_m,
        nc.semaphore("d_x0") as d_x0,
        nc.semaphore("d_x1") as d_x1,
        nc.semaphore("dsw") as dsw,
        nc.semaphore("d_strip") as d_strip,
        nc.semaphore("d_win0") as d_win0,
        nc.semaphore("d_win1") as d_win1,
        nc.semaphore("d_win2") as d_win2,
        nc.semaphore("d_comp0") as d_comp0,
        nc.semaphore("d_comp1") as d_comp1,
        nc.semaphore("pe") as pe_s,
        nc.semaphore("dve") as dve_s,
        nc.semaphore("act") as act_s,
        nc.semaphore("pool_id") as pool_id,
        nc.semaphore("pool_z") as pool_z,
        nc.semaphore("pool_zb") as pool_zb,
    ):
        d_x = [d_x0, d_x1]
        d_win = [d_win0, d_win1, d_win2]
        d_comp = [d_comp0, d_comp1]

        @block.gpsimd
        def _(g):
            g.memset(ident, 0.0).then_inc(pool_id, 1)
            g.wait_ge(pool_id, 1)
            g.affine_select(
                out=ident, in_=ident, compare_op=ALU.not_equal, fill=1.0,
                base=0, pattern=[[-1, P]], channel_multiplier=1,
            ).then_inc(pool_id, 1)
            g.memset(zero_t, 0.0).then_inc(pool_z, 1)
            g.memset(zbias, 0.0).then_inc(pool_zb, 1)
            bv_ap = bv_d[:]
            bv_bcast = bass.AP(
                tensor=bv_ap.tensor, offset=bv_ap.offset,
                ap=[[0, P], list(bv_ap.ap[0])],
            )
            g.dma_start(out=bv_b, in_=bv_bcast).then_inc(dsw, 16)

        @block.sync
        def _(sy):
            sy.dma_start(
                out=w_stage, in_=wq_d[:].rearrange("(c p) d -> p c d", p=P)
            ).then_inc(d_w, 16)
            with nc.allow_non_contiguous_dma(reason="512-element bias loads"):
                sy.dma_start(
                    out=bq_t, in_=bq_d[:].rearrange("(c p) -> p c", p=P)
                ).then_inc(d_b, 16)
                sy.dma_start(
                    out=bk_t, in_=bk_d[:].rearrange("(c p) -> p c", p=P)
                ).then_inc(d_b, 16)
            for b in range(NB):
                sy.dma_start(
                    out=mask_t[b][: PB[b], :],
                    in_=lm_d[I0[b] : I0[b] + PB[b], JBASE[b] : JEND[b]],
                ).then_inc(d_m, 16)
            sy.wait_ge(dve_s, mark["dve:wq"])
            sy.dma_start(
                out=w_stage, in_=wk_d[:].rearrange("(c p) d -> p c d", p=P)
            ).then_inc(d_w, 16)
            sy.wait_ge(dve_s, mark["dve:wk"])
            sy.dma_start(
                out=w_stage, in_=wv_d[:].rearrange("(c p) d -> p c d", p=P)
            ).then_inc(d_w, 16)
            for bi in range(B_PER_CORE):
                for sb in range(NB):
                    sy.dma_start(
                        out=x_sb[bi][sb][: PB[sb], :],
                        in_=x_d[bi, I0[sb] : I0[sb] + PB[sb], :],
                    ).then_inc(d_x[bi], 16)

        # ------------- PE stream -------------
        @block.tensor
        def _(te):
            te.wait_ge(pool_id, 2)
            for bi in range(B_PER_CORE):
                if bi > 0:
                    te.wait_ge(dve_s, dve_b_end[0])
                    te.wait_ge(act_s, act_b_end[0])
                te.wait_ge(d_x[bi], 128)
                # phase A: x^T transposes; bank k = 2c+h
                for c in range(4):
                    for h in range(2):
                        k = 2 * c + h
                        ins = None
                        for i4 in range(4):
                            sb = 4 * h + i4
                            pbk = PB[sb]
                            ins = te.matmul(
                                pb[k][:, 128 * i4 : 128 * i4 + pbk],
                                x_sb[bi][sb][:pbk, 128 * c : 128 * (c + 1)],
                                ident[:pbk, :pbk],
                                start=True, stop=True, is_transpose=True,
                            )
                        ins.then_inc(pe_s, 1)
                # phase B: QT(8) KT(8) V(8) groups
                for g in range(24):
                    bank = g % 8
                    if g < 8:
                        # covers all 4 xT chunk copybacks of this half AND the
                        # psum-bank WAR (cbA incs are in bank order)
                        te.wait_ge(dve_s, mark[f"dve:cbA{bi}_{6 + (g % 2)}"])
                    else:
                        te.wait_ge(dve_s, mark[f"dve:cbB{bi}_{g-8}"])
                    ins = None
                    if g < 16:
                        dc, h = divmod(g % 8, 2)
                        off, ln = S_HALF[h]
                        wt = wq_t if g < 8 else wk_t
                        for c in range(4):
                            ins = te.matmul(
                                pb[bank][:, :ln],
                                wt[:, c, 128 * dc : 128 * (dc + 1)],
                                xT[bi][:, c, off : off + ln],
                                start=(c == 0), stop=(c == 3),
                            )
                    else:
                        sb = g - 16
                        pbk = PB[sb]
                        for c in range(4):
                            ins = te.matmul(
                                pb[bank][:pbk, :],
                                xT[bi][:, c, I0[sb] : I0[sb] + pbk],
                                wv_t[:, c, :],
                                start=(c == 0), stop=(c == 3),
                            )
                    ins.then_inc(pe_s, 1)

                def emit_sc(b):
                    bank = b % 2
                    if b < 2:
                        te.wait_ge(dve_s, mark[f"dve:cbB{bi}_{16+bank}"])
                    else:
                        te.wait_ge(act_s, mark[f"act:exp2{bi}_{b-2}"])
                    pbk, we = PB[b], WE[b]
                    ins = None
                    for c in range(4):
                        ins = te.matmul(
                            pb[bank][:pbk, :we],
                            qT[:, c, I0[b] : I0[b] + pbk],
                            kT[:, c, JBASE[b] : JBASE[b] + we],
                            start=(c == 0), stop=(c == 3),
                        )
                    ins.then_inc(pe_s, 1)

                def emit_at(b):
                    bank = 2 + b % 2
                    if bi > 0 or b >= 2:
                        pbi, pbb = (bi, b - 2) if b >= 2 else (bi - 1, b + 6)
                        te.wait_ge(dve_s, mark[f"dve:atcb{pbi}_{pbb}"])
                    te.wait_ge(dve_s, mark[f"dve:stt{bi}_{b}"])
                    pbk = PB[b]
                    ins = None
                    for ti, (off, jb, wdt) in enumerate(CHUNKS[b]):
                        ins = te.matmul(
                            pb[bank][:wdt, ti * pbk : ti * pbk + pbk],
                            a_t[b % 3][:pbk, off : off + wdt],
                            ident[:pbk, :pbk],
                            start=True, stop=True, is_transpose=True,
                        )
                    ins.then_inc(pe_s, 1)

                def emit_cp(b):
                    bank = 4 + b % 2
                    # RAW on at_t (A^T copyback); also dominates the ccb WAR on
                    # this psum bank and the v_t copybacks in DVE stream order
                    te.wait_ge(dve_s, mark[f"dve:atcb{bi}_{b}"])
                    pbk = PB[b]
                    ins = None
                    nch = len(CHUNKS[b])
                    for ti, (off, jb, wdt) in enumerate(CHUNKS[b]):
                        ins = te.matmul(
                            pb[bank][:pbk, :],
                            at_t[b % 2][:wdt, ti * pbk : ti * pbk + pbk],
                            v_t[:wdt, jb, :],
                            start=(ti == 0), stop=(ti == nch - 1),
                        )
                    ins.then_inc(pe_s, 1)

                emit_sc(0)
                emit_sc(1)
                for b in range(NB):
                    emit_at(b)
                    emit_cp(b)
                    if b + 2 < NB:
                        emit_sc(b + 2)

        # ------------- ACT stream -------------
        @block.scalar
        def _(ac):
            ac.wait_ge(pool_zb, 1)
            for bi in range(B_PER_CORE):
                for b in range(NB):
                    pbk, wdt = PB[b], WB[b]
                    sc_p = pb[b % 2]
                    ac.wait_ge(pe_s, mark[f"pe:sc{bi}_{b}"])
                    if bi > 0 or b >= 2:
                        pbi, pbb = (bi, b - 2) if b >= 2 else (bi - 1, b + 6)
                        ac.wait_ge(dve_s, mark[f"dve:stt{pbi}_{pbb}"])
                    ac.activation(
                        out=e_t[b % 2][:pbk, :wdt], in_=sc_p[:pbk, :wdt],
                        func=AF.Exp, bias=zbias[:pbk, :], scale=float(SCALE),
                    ).then_inc(act_s, 1)
                    ac.activation(
                        out=e2_t[b % 2][:pbk, :wdt], in_=sc_p[:pbk, :wdt],
                        func=AF.Exp, bias=zbias[:pbk, :], scale=float(2.0 * SCALE),
                    ).then_inc(act_s, 1)
                    if b >= 1:
                        bb = b - 1
                        ac.wait_ge(dve_s, mark[f"dve:ttr{bi}_{bb}"])
                        ac.activation(
                            out=ssq_t[bb % 2][: PB[bb], :],
                            in_=ssq_t[bb % 2][: PB[bb], :],
                            func=AF.Sqrt, bias=zbias[: PB[bb], :], scale=1.0,
                        ).then_inc(act_s, 1)
                bb = NB - 1
                ac.wait_ge(dve_s, mark[f"dve:ttr{bi}_{bb}"])
                ac.activation(
                    out=ssq_t[bb % 2][: PB[bb], :], in_=ssq_t[bb % 2][: PB[bb], :],
                    func=AF.Sqrt, bias=zbias[: PB[bb], :], scale=1.0,
                ).then_inc(act_s, 1)

        # ------------- DVE stream -------------
        @block.vector
        def _(ve):
            ve.wait_ge(d_w, 16)
            ve.tensor_copy(out=wq_t, in_=w_stage).then_inc(dve_s, 1)
            ve.wait_ge(d_w, 32)
            ve.tensor_copy(out=wk_t, in_=w_stage).then_inc(dve_s, 1)
            ve.wait_ge(d_w, 48)
            ve.tensor_copy(out=wv_t, in_=w_stage).then_inc(dve_s, 1)
            ve.wait_ge(pool_z, 1)
            ve.tensor_copy(out=xT[0][:, :, S : S_PAD], in_=zero_t[:, 0:4]).then_inc(dve_s, 1)
            ve.tensor_copy(out=xT[1][:, :, S : S_PAD], in_=zero_t[:, 0:4]).then_inc(dve_s, 1)

            for bi in range(B_PER_CORE):
                for k in range(8):
                    c, h = divmod(k, 2)
                    off, ln = S_HALF[h]
                    cln = min(ln, S - off)  # copy only the transpose-produced cols
                    ve.wait_ge(pe_s, mark[f"pe:A{bi}_{k}"])
                    ve.tensor_copy(
                        out=xT[bi][:, c, off : off + cln], in_=pb[k][:, :cln]
                    ).then_inc(dve_s, 1)
                for g in range(24):
                    bank = g % 8
                    ve.wait_ge(pe_s, mark[f"pe:B{bi}_{g}"])
                    if g < 8:
                        if bi == 0 and g == 0:
                            ve.wait_ge(d_b, 32)
                        dc, h = divmod(g, 2)
                        off, ln = S_HALF[h]
                        ve.tensor_scalar(
                            out=qT[:, dc, off : off + ln], in0=pb[bank][:, :ln],
                            scalar1=bq_t[:, dc : dc + 1], scalar2=None, op0=ALU.add,
                        ).then_inc(dve_s, 1)
                    elif g < 16:
                        dc, h = divmod(g - 8, 2)
                        off, ln = S_HALF[h]
                        ve.tensor_scalar(
                            out=kT[:, dc, off : off + ln], in0=pb[bank][:, :ln],
                            scalar1=bk_t[:, dc : dc + 1], scalar2=None, op0=ALU.add,
                        ).then_inc(dve_s, 1)
                    else:
                        sb = g - 16
                        if bi == 0 and sb == 0:
                            ve.wait_ge(dsw, 16)
                        ve.tensor_add(
                            v_t[: PB[sb], sb, :], pb[bank][: PB[sb], :],
                            bv_b[: PB[sb], :],
                        ).then_inc(dve_s, 1)
                for b in range(NB + 2):
                    if b >= 2:
                        bb = b - 2
                        pbk = PB[bb]
                        nch = len(CHUNKS[bb])
                        ve.wait_ge(pe_s, mark[f"pe:at{bi}_{bb}"])
                        ve.tensor_copy(
                            out=at_t[bb % 2][:, : nch * pbk],
                            in_=pb[2 + bb % 2][:, : nch * pbk],
                        ).then_inc(dve_s, 1)
                        ve.wait_ge(pe_s, mark[f"pe:cp{bi}_{bb}"])
                        if bi > 0 or bb >= 2:
                            pbi, pbb = (bi, bb - 2) if bb >= 2 else (bi - 1, bb + 6)
                            ve.wait_ge(d_comp[bb % 2], mark[f"comp{bb % 2}:comp{pbi}_{pbb}"])
                        ve.tensor_copy(
                            out=cs_t[bb % 2][:pbk, :], in_=pb[4 + bb % 2][:pbk, :]
                        ).then_inc(dve_s, 1)
                    if b < NB:
                        pbk, wdt = PB[b], WB[b]
                        if bi == 0 and b == 0:
                            ve.wait_ge(d_m, 128)
                        ve.wait_ge(act_s, mark[f"act:exp2{bi}_{b}"])
                        ve.scalar_tensor_tensor(
                            out=scr_t[b % 2][:pbk, :wdt],
                            in0=e2_t[b % 2][:pbk, :wdt],
                            scalar=1.0,
                            in1=mask_t[b][:pbk, :wdt],
                            op0=ALU.mult, op1=ALU.mult,
                            accum_out=ssq_t[b % 2][:pbk, :],
                        ).then_inc(dve_s, 1)
                    if 1 <= b <= NB:
                        bb = b - 1
                        pbk, wdt = PB[bb], WB[bb]
                        ve.wait_ge(act_s, mark[f"act:sqrt{bi}_{bb}"])
                        ve.reciprocal(
                            out=r_t[bb % 2][:pbk, :], in_=ssq_t[bb % 2][:pbk, :]
                        ).then_inc(dve_s, 1)
                        ve.wait_ge(dve_s, mark[f"dve:recip{bi}_{bb}"])
                        if bi > 0 or bb >= 3:
                            pbi, pbb = (
                                (bi, bb - 3) if bb >= 3 else (bi - 1, [6, 7, 5][bb])
                            )
                            ve.wait_ge(d_win[bb % 3], mark[f"win{bb % 3}:win{pbi}_{pbb}"])
                        ve.scalar_tensor_tensor(
                            out=a_t[bb % 3][:pbk, :wdt],
                            in0=e_t[bb % 2][:pbk, :wdt],
                            scalar=r_t[bb % 2][:pbk, :],
                            in1=mask_t[bb][:pbk, :wdt],
                            op0=ALU.mult, op1=ALU.mult,
                        ).then_inc(dve_s, 1)

        # ------------- output DMA stream -------------
        @block.sync
        def _(sy):
            sy.wait_ge(pool_z, 1)
            for bi in range(B_PER_CORE):
                for b in range(NB):
                    pbk, wdt = PB[b], WB[b]
                    sy.wait_ge(dve_s, mark[f"dve:stt{bi}_{b}"])
                    sy.dma_start(
                        out=attn_d[bi, I0[b] : I0[b] + pbk, JBASE[b] : JEND[b]],
                        in_=a_t[b % 3][:pbk, :wdt],
                    ).then_inc(d_win[b % 3], 16)
                    if JBASE[b] > 0:
                        sy.dma_start(
                            out=attn_d[bi, I0[b] : I0[b] + pbk, 0 : JBASE[b]],
                            in_=zero_t[:pbk, : JBASE[b]],
                        ).then_inc(d_strip, 16)
                    if JEND[b] < S:
                        sy.dma_start(
                            out=attn_d[bi, I0[b] : I0[b] + pbk, JEND[b] : S],
                            in_=zero_t[:pbk, : S - JEND[b]],
                        ).then_inc(d_strip, 16)
                    sy.wait_ge(dve_s, mark[f"dve:ccb{bi}_{b}"])
                    sy.dma_start(
                        out=comp_d[bi, I0[b] : I0[b] + pbk, :],
                        in_=cs_t[b % 2][:pbk, :],
                    ).then_inc(d_comp[b % 2], 16)
            sy.wait_ge(d_win[0], cnt["win0"])
            sy.wait_ge(d_win[1], cnt["win1"])
            sy.wait_ge(d_win[2], cnt["win2"])
            sy.wait_ge(d_comp[0], cnt["comp0"])
            sy.wait_ge(d_comp[1], cnt["comp1"])
            sy.wait_ge(d_strip, cnt["strip"])

    return nc


_PROGRAM = None


def _get_program():
    global _PROGRAM
    if _PROGRAM is None:
        _PROGRAM = build_program()
    return _PROGRAM


def kernel(x, Wq, bq, Wk, bk, Wv, bv, local_mat):
    nc = _get_program()
    x = np.ascontiguousarray(np.asarray(x, dtype=np.float32))
    shared = {
        "Wq": np.asarray(Wq, np.float32), "bq": np.asarray(bq, np.float32),
        "Wk": np.asarray(Wk, np.float32), "bk": np.asarray(bk, np.float32),
        "Wv": np.asarray(Wv, np.float32), "bv": np.asarray(bv, np.float32),
        "local_mat": np.ascontiguousarray(np.asarray(local_mat, np.float32)),
    }
    in_maps = [
        {"x": x[B_PER_CORE * c : B_PER_CORE * (c + 1)], **shared}
        for c in range(N_CORES)
    ]
    res = run_bass_kernel_spmd(nc, in_maps, list(range(N_CORES)))
    comp = np.concatenate([res.results[c]["comp"] for c in range(N_CORES)], axis=0)
    attn = np.concatenate([res.results[c]["attn"] for c in range(N_CORES)], axis=0)
    return comp, attn


if __name__ == "__main__":
    build_program()
    print("program built OK")


# revision 37
# speedup vs baseline: 36078.7329x; 36078.7329x over previous
"""Banded local-attention kernel for Trainium2, 8 NeuronCores, raw-bass SPMD.

Math: reference computes attn = softmax(scores); attn_local = attn*mask;
attn_final = attn_local / ||attn_local||_2(row). The softmax denominator Z
cancels in the L2 normalization, so:
    attn_final[i,j] = mask[i,j]*exp(s[i,j]) / sqrt(sum_k mask[i,k]*exp(2*s[i,k]))
Only the banded window of scores (|i-j| <= 50) is ever needed.

Sharding: batch (16) over 8 cores, 2 batches/core. Per batch:
  x^T via PE transposes -> Q^T,K^T (transposed layout) + V (natural) via f32r
  matmuls -> banded scores per 128-row block -> exp/exp2 on ACT -> masked
  sum-of-squares + rsqrt scaling on DVE -> attn window written + zero strips
  -> A^T via PE transposes -> compound = A @ V.

Semaphore discipline (1 wait per instruction; DMA waits must equal the
total of all DMAs issued on that semaphore at satisfaction time):
  d_w: weight stage loads (serialized via dve gates) ; d_b: bq+bk ;
  d_m: masks ; d_x0/d_x1: per-batch x tiles ; dsw: bv broadcast (SWDGE) ;
  d_win[3]/d_comp[2]: per-SBUF-slot output DMAs ; d_strip: zero strips ;
  pe/dve/act: in-order engine progress ; pool_*: gpsimd objects.
"""
import sys
import numpy as np

sys.path.insert(0, "/opt/trn_rl_repo")

import concourse.bass as bass  # noqa: E402
from concourse import mybir  # noqa: E402
from concourse.bass_utils import run_bass_kernel_spmd  # noqa: E402

F32 = mybir.dt.float32
F32R = mybir.dt.float32r
AF = mybir.ActivationFunctionType
ALU = mybir.AluOpType

S = 1001
D = 512
P = 128
NB = 8
B_PER_CORE = 2
N_CORES = 8
SCALE = 1.0 / float(np.sqrt(512.0))

PB = [128] * 7 + [105]
I0 = [128 * b for b in range(NB)]
LOBLK = [max(0, b - 1) for b in range(NB)]
HIBLK = [min(NB - 1, b + 1) for b in range(NB)]
JBASE = [128 * LOBLK[b] for b in range(NB)]
JEND = [min(S, 128 * (HIBLK[b] + 1)) for b in range(NB)]
WB = [JEND[b] - JBASE[b] for b in range(NB)]
CHUNKS = [
    [(128 * (jb - LOBLK[b]), jb, PB[jb]) for jb in range(LOBLK[b], HIBLK[b] + 1)]
    for b in range(NB)
]
S_HALF = [(0, 512), (512, 490)]   # halves of padded S (1002) for f32r even-N
S_PAD = 1002
WE = [w + (w % 2) for w in WB]     # even-padded score window widths


def _counts(niter):
    """Dry pass: cumulative semaphore counts for every labeled event."""
    cnt = {"pe": 0, "dve": 0, "act": 0, "plm": 0,
           "win0": 0, "win1": 0, "win2": 0, "comp0": 0, "comp1": 0,
           "strip": 0}
    mark = {}

    def rec(sem, label, inc=1):
        cnt[sem] += inc
        mark[f"{sem}:{label}"] = cnt[sem]

    rec("dve", "wq")
    rec("dve", "wk")
    rec("dve", "pad0")
    rec("dve", "pad1")

    PH_A = [6, 0, 2, 4, 7, 1, 3, 5]   # h=0 banks first, earliest-free first
    for bi in range(niter):
        for k in PH_A:
            rec("pe", f"A{bi}_{k}")
        for k in PH_A:
            rec("dve", f"cbA{bi}_{k}")
        if bi == 0:
            rec("dve", "wv")
        for g in range(16):
            rec("pe", f"B{bi}_{g}")
        for g in range(16):
            rec("dve", f"cbB{bi}_{g}")
        rec("pe", f"sc{bi}_0")
        rec("pe", f"sc{bi}_1")
        rec("pe", f"sc{bi}_2")
        for g in range(16, 24):
            rec("pe", f"B{bi}_{g}")
        for b in range(NB):
            rec("pe", f"at{bi}_{b}")
            rec("pe", f"cp{bi}_{b}")
            if b + 3 < NB:
                rec("pe", f"sc{bi}_{b+3}")
        for b in range(NB + 1):
            if b < NB:
                rec("act", f"exp{bi}_{b}")
            if b >= 1:
                rec("act", f"sqrt{bi}_{b-1}")
        for b in range(NB):
            rec("plm", f"em{bi}_{b}")
        for g in range(16, 24):
            rec("dve", f"cbB{bi}_{g}")
        for b in range(NB + 2):
            if b >= 2:
                rec("dve", f"atcb{bi}_{b-2}")
                rec("dve", f"ccb{bi}_{b-2}")
            if b < NB:
                rec("dve", f"sq{bi}_{b}")
            if 1 <= b <= NB:
                rec("dve", f"recip{bi}_{b-1}")
                rec("dve", f"stt{bi}_{b-1}")
        for b in range(NB):
            rec(f"win{b % 3}", f"win{bi}_{b}", 16)
            if JBASE[b] > 0:
                rec("strip", f"ls{bi}_{b}", 16)
            if JEND[b] < S:
                rec("strip", f"rs{bi}_{b}", 16)
            rec(f"comp{b % 2}", f"comp{bi}_{b}", 16)
    return cnt, mark


def build_program(niter=B_PER_CORE):
    cnt, mark = _counts(niter)
    nc = bass.Bass()

    x_d = nc.declare_dram_parameter("x", [B_PER_CORE, S, D], F32, isOutput=False)
    wq_d = nc.declare_dram_parameter("Wq", [D, D], F32, isOutput=False)
    bq_d = nc.declare_dram_parameter("bq", [D], F32, isOutput=False)
    wk_d = nc.declare_dram_parameter("Wk", [D, D], F32, isOutput=False)
    bk_d = nc.declare_dram_parameter("bk", [D], F32, isOutput=False)
    wv_d = nc.declare_dram_parameter("Wv", [D, D], F32, isOutput=False)
    bv_d = nc.declare_dram_parameter("bv", [D], F32, isOutput=False)
    lm_d = nc.declare_dram_parameter("local_mat", [S, S], F32, isOutput=False)
    comp_d = nc.declare_dram_parameter("comp", [B_PER_CORE, S, D], F32, isOutput=True)
    attn_d = nc.declare_dram_parameter("attn", [B_PER_CORE, S, S], F32, isOutput=True)

    t = nc.alloc_sbuf_tensor
    w_stage = t("w_stage", [P, 4, D], F32).ap()
    w_stage2 = t("w_stage2", [P, 4, D], F32).ap()
    w_stage3 = t("w_stage3", [P, 4, D], F32).ap()
    wq_t = t("wq_t", [P, 4, D], F32R).ap()
    wk_t = t("wk_t", [P, 4, D], F32R).ap()
    wv_t = t("wv_t", [P, 4, D], F32R).ap()
    bq_t = t("bq_t", [P, 4], F32).ap()
    bk_t = t("bk_t", [P, 4], F32).ap()
    bv_b = t("bv_b", [P, D], F32).ap()
    mask_t = [t(f"mask{b}", [P, WB[b]], F32).ap() for b in range(NB)]
    zero_t = t("zero_t", [P, 768], F32).ap()
    ident = t("ident", [P, P], F32).ap()
    zbias = t("zbias", [P, 1], F32).ap()

    x_sb = [t(f"x_{bi}", [P, NB, D], F32).ap() for bi in range(B_PER_CORE)]
    xT = [t(f"xT_{bi}", [P, 4, S_PAD], F32R).ap() for bi in range(B_PER_CORE)]
    qT = t("qT", [P, 4, S_PAD], F32R).ap()
    kT = t("kT", [P, 4, S_PAD], F32R).ap()
    v_t = t("v_t", [P, NB, D], F32R).ap()

    e_t = [t(f"e_{i}", [P, 384], F32).ap() for i in range(3)]
    em_t = [t(f"em_{i}", [P, 384], F32).ap() for i in range(3)]
    a_t = [t(f"a_{i}", [P, 384], F32).ap() for i in range(3)]
    at_t = [t(f"at_{i}", [P, 384], F32R).ap() for i in range(2)]
    cs_t = [t(f"cs_{i}", [P, D], F32).ap() for i in range(2)]
    ssq_t = [t(f"ssq_{i}", [P, 1], F32).ap() for i in range(3)]
    r_t = [t(f"r_{i}", [P, 1], F32).ap() for i in range(3)]

    pb = [nc.alloc_psum_tensor(f"pb{i}", [P, D], F32).ap() for i in range(8)]

    with (
        nc.Block() as block,
        nc.semaphore("d_w") as d_w,
        nc.semaphore("d_w1") as d_w1,
        nc.semaphore("d_w2") as d_w2,
        nc.semaphore("d_b") as d_b,
        nc.semaphore("d_m") as d_m,
        nc.semaphore("d_x0") as d_x0,
        nc.semaphore("d_x1") as d_x1,
        nc.semaphore("dsw") as dsw,
        nc.semaphore("d_strip") as d_strip,
        nc.semaphore("d_win0") as d_win0,
        nc.semaphore("d_win1") as d_win1,
        nc.semaphore("d_win2") as d_win2,
        nc.semaphore("d_comp0") as d_comp0,
        nc.semaphore("d_comp1") as d_comp1,
        nc.semaphore("pe") as pe_s,
        nc.semaphore("dve") as dve_s,
        nc.semaphore("act") as act_s,
        nc.semaphore("pool_id") as pool_id,
        nc.semaphore("pool_z") as pool_z,
        nc.semaphore("pool_zb") as pool_zb,
        nc.semaphore("plm") as plm_s,
    ):
        d_x = [d_x0, d_x1]
        d_xb = [d_x0b, d_x1b]
        d_win = [d_win0, d_win1, d_win2]
        d_comp = [d_comp0, d_comp1]

        @block.gpsimd
        def _(g):
            g.memset(ident, 0.0).then_inc(pool_id, 1)
            g.wait_ge(pool_id, 1)
            g.affine_select(
                out=ident, in_=ident, compare_op=ALU.not_equal, fill=1.0,
                base=0, pattern=[[-1, P]], channel_multiplier=1,
            ).then_inc(pool_id, 1)
            g.memset(zero_t, 0.0).then_inc(pool_z, 1)
            g.memset(zbias, 0.0).then_inc(pool_zb, 1)
            bv_ap = bv_d[:]
            bv_bcast = bass.AP(
                tensor=bv_ap.tensor, offset=bv_ap.offset,
                ap=[[0, P], list(bv_ap.ap[0])],
            )
            g.dma_start(out=bv_b, in_=bv_bcast).then_inc(dsw, 16)
            g.wait_ge(pool_z, 1)
            for bi in range(niter):
                bx = bi % 2
                for b in range(NB):
                    pbk, wdt = PB[b], WB[b]
                    if bi == 0 and b == 0:
                        g.wait_ge(d_m, 128)
                    g.wait_ge(act_s, mark[f"act:exp{bi}_{b}"])
                    g.tensor_mul(
                        em_t[b % 3][:pbk, :wdt],
                        e_t[b % 3][:pbk, :wdt],
                        mask_t[b][:pbk, :wdt],
                    ).then_inc(plm_s, 1)
                    g.wait_ge(plm_s, mark[f"plm:em{bi}_{b}"])
                    if JBASE[b] > 0:
                        g.dma_start(
                            out=attn_d[bx, I0[b] : I0[b] + pbk, 0 : JBASE[b]],
                            in_=zero_t[:pbk, : JBASE[b]],
                        ).then_inc(d_strip, 16)
                    if JEND[b] < S:
                        g.dma_start(
                            out=attn_d[bx, I0[b] : I0[b] + pbk, JEND[b] : S],
                            in_=zero_t[:pbk, : S - JEND[b]],
                        ).then_inc(d_strip, 16)

        @block.sync
        def _(sy):
            sy.dma_start(
                out=w_stage, in_=wq_d[:].rearrange("(c p) d -> p c d", p=P)
            ).then_inc(d_w, 16)
            sy.dma_start(
                out=w_stage2, in_=wk_d[:].rearrange("(c p) d -> p c d", p=P)
            ).then_inc(d_w1, 16)
            sy.dma_start(
                out=w_stage3, in_=wv_d[:].rearrange("(c p) d -> p c d", p=P)
            ).then_inc(d_w2, 16)
            with nc.allow_non_contiguous_dma(reason="512-element bias loads"):
                sy.dma_start(
                    out=bq_t, in_=bq_d[:].rearrange("(c p) -> p c", p=P)
                ).then_inc(d_b, 16)
                sy.dma_start(
                    out=bk_t, in_=bk_d[:].rearrange("(c p) -> p c", p=P)
                ).then_inc(d_b, 16)
            for b in range(NB):
                sy.dma_start(
                    out=mask_t[b][: PB[b], :],
                    in_=lm_d[I0[b] : I0[b] + PB[b], JBASE[b] : JEND[b]],
                ).then_inc(d_m, 16)


        # ------------- PE stream -------------
        @block.tensor
        def _(te):
            te.wait_ge(pool_id, 2)
            PH_A = [6, 0, 2, 4, 7, 1, 3, 5]
            # last tenant of each psum bank in batch bi-1's phase C
            bank_free = {
                0: ("act", "exp", 6), 1: ("act", "exp", 7), 6: ("act", "exp", 5),
                2: ("dve", "atcb", 6), 3: ("dve", "atcb", 7),
                4: ("dve", "ccb", 6), 5: ("dve", "ccb", 7),
                7: ("dve", "cbB", 20),
            }
            for bi in range(niter):
                bx = bi % 2
                # phase A: x^T transposes into banks, h=0 half first
                for ki, k in enumerate(PH_A):
                    c, h = divmod(k, 2)
                    if bi < 2 and ki == 0:
                        te.wait_ge(d_x[bx], 16)
                    if bi < 2 and ki == 4:
                        te.wait_ge(d_xb[bx], 32)
                    if bi > 0:
                        sem, kind, blk = bank_free[k]
                        te.wait_ge(
                            dve_s if sem == "dve" else act_s,
                            mark[f"{sem}:{kind}{bi-1}_{blk}"],
                        )
                    ins = None
                    for i4 in range(4):
                        sb = 4 * h + i4
                        pbk = PB[sb]
                        ins = te.matmul(
                            pb[k][:, 128 * i4 : 128 * i4 + pbk],
                            x_sb[bx][:pbk, sb, 128 * c : 128 * (c + 1)],
                            ident[:pbk, :pbk],
                            start=True, stop=True, is_transpose=True,
                        )
                    ins.then_inc(pe_s, 1)
                # phase B: QT(8) KT(8) V(8) groups
                def emit_qk(g):
                    bank = g % 8
                    if g < 8:
                        # covers all 4 xT chunk copybacks of this half AND the
                        # psum-bank WAR (banks 6,5 are the last h0/h1 in PH_A)
                        te.wait_ge(dve_s, mark[f"dve:cbA{bi}_{4 + (g % 2)}"])
                    else:
                        te.wait_ge(dve_s, mark[f"dve:cbB{bi}_{g-8}"])
                    dc, h = divmod(g % 8, 2)
                    off, ln = S_HALF[h]
                    wt = wq_t if g < 8 else wk_t
                    ins = None
                    for c in range(4):
                        ins = te.matmul(
                            pb[bank][:, :ln],
                            wt[:, c, 128 * dc : 128 * (dc + 1)],
                            xT[bx][:, c, off : off + ln],
                            start=(c == 0), stop=(c == 3),
                        )
                    ins.then_inc(pe_s, 1)

                VBANKS = [2, 3, 4, 5, 7, 2, 3, 4]
                VWAITS = [10, 11, 12, 13, 15, 16, 17, 18]

                def emit_v(i):
                    bank = VBANKS[i]
                    te.wait_ge(dve_s, mark[f"dve:cbB{bi}_{VWAITS[i]}"])
                    pbk = PB[i]
                    ins = None
                    for c in range(4):
                        ins = te.matmul(
                            pb[bank][:pbk, :],
                            xT[bx][:, c, I0[i] : I0[i] + pbk],
                            wv_t[:, c, :],
                            start=(c == 0), stop=(c == 3),
                        )
                    ins.then_inc(pe_s, 1)

                def emit_sc(b):
                    bank = [0, 1, 6][b % 3]
                    if b < 3:
                        # needs ALL dc chunks of qT/kT half 0: last is KT g14;
                        # also covers the bank tenants (KT g8,g9,g14)
                        te.wait_ge(dve_s, mark[f"dve:cbB{bi}_14"])
                    else:
                        te.wait_ge(act_s, mark[f"act:exp{bi}_{b-3}"])
                    pbk, we = PB[b], WE[b]
                    ins = None
                    for c in range(4):
                        ins = te.matmul(
                            pb[bank][:pbk, :we],
                            qT[:, c, I0[b] : I0[b] + pbk],
                            kT[:, c, JBASE[b] : JBASE[b] + we],
                            start=(c == 0), stop=(c == 3),
                        )
                    ins.then_inc(pe_s, 1)

                def emit_at(b):
                    bank = 2 + b % 2
                    te.wait_ge(dve_s, mark[f"dve:stt{bi}_{b}"])
                    pbk = PB[b]
                    ins = None
                    for ti, (off, jb, wdt) in enumerate(CHUNKS[b]):
                        ins = te.matmul(
                            pb[bank][:wdt, ti * pbk : ti * pbk + pbk],
                            a_t[b % 3][:pbk, off : off + wdt],
                            ident[:pbk, :pbk],
                            start=True, stop=True, is_transpose=True,
                        )
                    ins.then_inc(pe_s, 1)

                def emit_cp(b):
                    bank = 4 + b % 2
                    # RAW on at_t (A^T copyback); also dominates the ccb WAR on
                    # this psum bank and the v_t copybacks in DVE stream order
                    te.wait_ge(dve_s, mark[f"dve:atcb{bi}_{b}"])
                    pbk = PB[b]
                    ins = None
                    nch = len(CHUNKS[b])
                    for ti, (off, jb, wdt) in enumerate(CHUNKS[b]):
                        ins = te.matmul(
                            pb[bank][:pbk, :],
                            at_t[b % 2][:wdt, ti * pbk : ti * pbk + pbk],
                            v_t[:wdt, jb, :],
                            start=(ti == 0), stop=(ti == nch - 1),
                        )
                    ins.then_inc(pe_s, 1)

                for g in range(16):
                    emit_qk(g)
                emit_sc(0)
                emit_sc(1)
                emit_sc(2)
                for i in range(8):
                    emit_v(i)
                for b in range(NB):
                    emit_at(b)
                    emit_cp(b)
                    if b + 3 < NB:
                        emit_sc(b + 3)

        # ------------- ACT stream -------------
        @block.scalar
        def _(ac):
            for bi in range(B_PER_CORE):
                ac.dma_start(
                    out=x_sb[bi][:, 0:4, :],
                    in_=x_d[bi, 0 : 4 * P, :].rearrange("(sb p) d -> p sb d", p=P),
                ).then_inc(d_x[bi], 16)
                ac.dma_start(
                    out=x_sb[bi][:, 4:7, :],
                    in_=x_d[bi, 4 * P : 7 * P, :].rearrange("(sb p) d -> p sb d", p=P),
                ).then_inc(d_xb[bi], 16)
                ac.dma_start(
                    out=x_sb[bi][:105, 7, :], in_=x_d[bi, 7 * P : S, :]
                ).then_inc(d_xb[bi], 16)
            ac.wait_ge(pool_zb, 1)
            for bi in range(niter):
                for b in range(NB + 1):
                    if b < NB:
                        pbk, wdt = PB[b], WB[b]
                        sc_p = pb[[0, 1, 6][b % 3]]
                        ac.wait_ge(pe_s, mark[f"pe:sc{bi}_{b}"])
                        # WAR: e slot reuse (sq of b-3 wrote it last)
                        if bi > 0 or b >= 3:
                            pbi, pbb = (bi, b - 3) if b >= 3 else (bi - 1, [6, 7, 5][b])
                            ac.wait_ge(dve_s, mark[f"dve:sq{pbi}_{pbb}"])
                        ac.activation(
                            out=e_t[b % 3][:pbk, :wdt], in_=sc_p[:pbk, :wdt],
                            func=AF.Exp, bias=zbias[:pbk, :], scale=float(SCALE),
                        ).then_inc(act_s, 1)
                    if b >= 1:
                        bb = b - 1
                        ac.wait_ge(dve_s, mark[f"dve:sq{bi}_{bb}"])
                        ac.activation(
                            out=ssq_t[bb % 3][: PB[bb], :], in_=ssq_t[bb % 3][: PB[bb], :],
                            func=AF.Sqrt, bias=zbias[: PB[bb], :], scale=1.0,
                        ).then_inc(act_s, 1)

        # ------------- DVE stream -------------
        @block.vector
        def _(ve):
            ve.wait_ge(d_w, 16)
            ve.tensor_copy(out=wq_t, in_=w_stage).then_inc(dve_s, 1)
            ve.wait_ge(d_w1, 16)
            ve.tensor_copy(out=wk_t, in_=w_stage2).then_inc(dve_s, 1)
            ve.wait_ge(pool_z, 1)
            ve.tensor_copy(out=xT[0][:, :, S : S_PAD], in_=zero_t[:, 0:4]).then_inc(dve_s, 1)
            ve.tensor_copy(out=xT[1][:, :, S : S_PAD], in_=zero_t[:, 0:4]).then_inc(dve_s, 1)

            PH_A = [6, 0, 2, 4, 7, 1, 3, 5]
            for bi in range(niter):
                bx = bi % 2
                for k in PH_A:
                    c, h = divmod(k, 2)
                    off, ln = S_HALF[h]
                    cln = min(ln, S - off)  # copy only the transpose-produced cols
                    ve.wait_ge(pe_s, mark[f"pe:A{bi}_{k}"])
                    ve.tensor_copy(
                        out=xT[bx][:, c, off : off + cln], in_=pb[k][:, :cln]
                    ).then_inc(dve_s, 1)
                if bi == 0:
                    ve.wait_ge(d_w2, 16)
                    ve.tensor_copy(out=wv_t, in_=w_stage3).then_inc(dve_s, 1)
                for g in range(16):
                    bank = g % 8
                    ve.wait_ge(pe_s, mark[f"pe:B{bi}_{g}"])
                    if g < 8:
                        if bi == 0 and g == 0:
                            ve.wait_ge(d_b, 32)
                        dc, h = divmod(g, 2)
                        off, ln = S_HALF[h]
                        ve.tensor_scalar(
                            out=qT[:, dc, off : off + ln], in0=pb[bank][:, :ln],
                            scalar1=bq_t[:, dc : dc + 1], scalar2=None, op0=ALU.add,
                        ).then_inc(dve_s, 1)
                    else:
                        dc, h = divmod(g - 8, 2)
                        off, ln = S_HALF[h]
                        ve.tensor_scalar(
                            out=kT[:, dc, off : off + ln], in0=pb[bank][:, :ln],
                            scalar1=bk_t[:, dc : dc + 1], scalar2=None, op0=ALU.add,
                        ).then_inc(dve_s, 1)
                VBANKS = [2, 3, 4, 5, 7, 2, 3, 4]
                for i in range(8):
                    ve.wait_ge(pe_s, mark[f"pe:B{bi}_{16+i}"])
                    if bi == 0 and i == 0:
                        ve.wait_ge(dsw, 16)
                    ve.tensor_add(
                        v_t[: PB[i], i, :], pb[VBANKS[i]][: PB[i], :],
                        bv_b[: PB[i], :],
                    ).then_inc(dve_s, 1)
                for b in range(NB + 2):
                    if b >= 2:
                        bb = b - 2
                        pbk = PB[bb]
                        nch = len(CHUNKS[bb])
                        ve.wait_ge(pe_s, mark[f"pe:at{bi}_{bb}"])
                        ve.tensor_copy(
                            out=at_t[bb % 2][:, : nch * pbk],
                            in_=pb[2 + bb % 2][:, : nch * pbk],
                        ).then_inc(dve_s, 1)
                        ve.wait_ge(pe_s, mark[f"pe:cp{bi}_{bb}"])
                        if bi > 0 or bb >= 2:
                            pbi, pbb = (bi, bb - 2) if bb >= 2 else (bi - 1, bb + 6)
                            ve.wait_ge(d_comp[bb % 2], mark[f"comp{bb % 2}:comp{pbi}_{pbb}"])
                        ve.tensor_copy(
                            out=cs_t[bb % 2][:pbk, :], in_=pb[4 + bb % 2][:pbk, :]
                        ).then_inc(dve_s, 1)
                    if b < NB:
                        pbk, wdt = PB[b], WB[b]
                        ve.wait_ge(plm_s, mark[f"plm:em{bi}_{b}"])
                        ve.scalar_tensor_tensor(
                            out=e_t[b % 3][:pbk, :wdt],
                            in0=em_t[b % 3][:pbk, :wdt],
                            scalar=1.0,
                            in1=em_t[b % 3][:pbk, :wdt],
                            op0=ALU.mult, op1=ALU.mult,
                            accum_out=ssq_t[b % 3][:pbk, :],
                        ).then_inc(dve_s, 1)
                    if 1 <= b <= NB:
                        bb = b - 1
                        pbk, wdt = PB[bb], WB[bb]
                        ve.wait_ge(act_s, mark[f"act:sqrt{bi}_{bb}"])
                        ve.reciprocal(
                            out=r_t[bb % 3][:pbk, :], in_=ssq_t[bb % 3][:pbk, :]
                        ).then_inc(dve_s, 1)
                        ve.wait_ge(dve_s, mark[f"dve:recip{bi}_{bb}"])
                        if bi > 0 or bb >= 3:
                            pbi, pbb = (
                                (bi, bb - 3) if bb >= 3 else (bi - 1, [6, 7, 5][bb])
                            )
                            ve.wait_ge(d_win[bb % 3], mark[f"win{bb % 3}:win{pbi}_{pbb}"])
                        ve.tensor_scalar(
                            out=a_t[bb % 3][:pbk, :wdt],
                            in0=em_t[bb % 3][:pbk, :wdt],
                            scalar1=r_t[bb % 3][:pbk, :], scalar2=None,
                            op0=ALU.mult,
                        ).then_inc(dve_s, 1)

        # ------------- output DMA stream -------------
        @block.sync
        def _(sy):
            for bi in range(niter):
                bx = bi % 2
                for b in range(NB):
                    pbk, wdt = PB[b], WB[b]
                    sy.wait_ge(dve_s, mark[f"dve:stt{bi}_{b}"])
                    sy.dma_start(
                        out=attn_d[bx, I0[b] : I0[b] + pbk, JBASE[b] : JEND[b]],
                        in_=a_t[b % 3][:pbk, :wdt],
                    ).then_inc(d_win[b % 3], 16)
                    sy.wait_ge(dve_s, mark[f"dve:ccb{bi}_{b}"])
                    sy.dma_start(
                        out=comp_d[bx, I0[b] : I0[b] + pbk, :],
                        in_=cs_t[b % 2][:pbk, :],
                    ).then_inc(d_comp[b % 2], 16)
            sy.wait_ge(d_win[0], cnt["win0"])
            sy.wait_ge(d_win[1], cnt["win1"])
            sy.wait_ge(d_win[2], cnt["win2"])
            sy.wait_ge(d_comp[0], cnt["comp0"])
            sy.wait_ge(d_comp[1], cnt["comp1"])
            sy.wait_ge(d_strip, cnt["strip"])

    return nc


_PROGRAM = None


def _get_program():
    global _PROGRAM
    if _PROGRAM is None:
        _PROGRAM = build_program()
    return _PROGRAM


def kernel(x, Wq, bq, Wk, bk, Wv, bv, local_mat):
    nc = _get_program()
    x = np.ascontiguousarray(np.asarray(x, dtype=np.float32))
    shared = {
        "Wq": np.asarray(Wq, np.float32), "bq": np.asarray(bq, np.float32),
        "Wk": np.asarray(Wk, np.float32), "bk": np.asarray(bk, np.float32),
        "Wv": np.asarray(Wv, np.float32), "bv": np.asarray(bv, np.float32),
        "local_mat": np.ascontiguousarray(np.asarray(local_mat, np.float32)),
    }
    in_maps = [
        {"x": x[B_PER_CORE * c : B_PER_CORE * (c + 1)], **shared}
        for c in range(N_CORES)
    ]
    res = run_bass_kernel_spmd(nc, in_maps, list(range(N_CORES)))
    comp = np.concatenate([res.results[c]["comp"] for c in range(N_CORES)], axis=0)
    attn = np.concatenate([res.results[c]["attn"] for c in range(N_CORES)], axis=0)
    return comp, attn


if __name__ == "__main__":
    build_program()
    print("program built OK")
